# revision 5
# baseline (speedup 1.0000x reference)
"""Trainium2 8-core Bass kernel for fused-QKV causal attention.

Problem: q,k,v [2,2048,1024] f32; fused qkv_w [3072,1024]; proj_w [1024,1024];
returns (x [2,2048,1024], attn [2,16,2048,2048]) matching the jax reference.

Sharding: 8 cores = 2 batches x 4 head-groups (4 heads each). Host transposes
activations/weights so device DMAs are contiguous; host sums the 4 per-core
output-projection partials per batch (the only cross-core reduction).

Matmul inputs are bf16 (PE runs 1 cyc/row vs fp32's 2x2 half-speed passes);
all accumulation/softmax statistics stay fp32.
"""
import os
import sys

sys.path.insert(0, "/opt/trn_rl_repo")

# The image's antenv lacks axon_hooks; fabricate it so trace=True works.
import types as _types

if "antenv.axon_hooks" not in sys.modules:
    _hooks = _types.ModuleType("antenv.axon_hooks")
    _h = [None]
    _hooks.set_axon_ntff_profile_hook = lambda h: _h.__setitem__(0, h)
    _hooks.get_axon_ntff_profile_hook = lambda: _h[0]
    sys.modules["antenv.axon_hooks"] = _hooks
    try:
        from trn_agent_boot.trn_boot import _ntff_profile_via_ctypes

        _hooks.set_axon_ntff_profile_hook(
            _ntff_profile_via_ctypes("/opt/axon/libaxon_pjrt.so")
        )
    except Exception:
        pass

import ml_dtypes
import numpy as np

import concourse.bass as bass
import concourse.mybir as mybir
import concourse.tile as tile
from concourse import bacc
from concourse.bass_utils import run_bass_kernel_spmd
from concourse.masks import make_identity

F32 = mybir.dt.float32
BF16 = mybir.dt.bfloat16
AX = mybir.AxisListType
ALU = mybir.AluOpType
ACTF = mybir.ActivationFunctionType
NPBF = ml_dtypes.bfloat16

B, N, C = 2, 2048, 1024
H, HD = 16, 64
SCALE = HD ** -0.5
HPC = 4            # heads per core
N_CORES = 8
NB = N // 128      # 16 query/key blocks
CC = C // 128      # 8 contraction chunks
MASK_VAL = -30000.0
ATTN_OUT_BF16 = True   # device writes attn probs as bf16; host casts to f32

_cache = {}
last_exec_time_ns = None
last_results = None


def _ceil_div(a, b):
    return (a + b - 1) // b


def build_kernel(use_mask: bool):
    import time as _time
    _t0 = _time.time()
    nc = bacc.Bacc("TRN2", target_bir_lowering=False, debug=False,
                   num_devices=N_CORES)

    DT = BF16
    qT = nc.dram_tensor("qT", [C, N], DT, kind="ExternalInput").ap()
    kT = nc.dram_tensor("kT", [C, N], DT, kind="ExternalInput").ap()
    vT = nc.dram_tensor("vT", [C, N], DT, kind="ExternalInput").ap()
    wqT = nc.dram_tensor("wqT", [C, HPC * HD], DT, kind="ExternalInput").ap()
    wkT = nc.dram_tensor("wkT", [C, HPC * HD], DT, kind="ExternalInput").ap()
    wvT = nc.dram_tensor("wvT", [C, HPC * HD], DT, kind="ExternalInput").ap()
    pjT = nc.dram_tensor("pjT", [HPC * HD, C], DT, kind="ExternalInput").ap()
    mb_d = nc.dram_tensor("maskb", [128, 128], F32, kind="ExternalInput").ap()
    mbT_d = nc.dram_tensor("maskbT", [128, 128], F32, kind="ExternalInput").ap()

    ADT = BF16 if ATTN_OUT_BF16 else F32
    attn_o = nc.dram_tensor("attn", [HPC, N, N], ADT, kind="ExternalOutput").ap()
    x_o = nc.dram_tensor("x", [N, C], F32, kind="ExternalOutput").ap()

    with tile.TileContext(nc) as tc:
        from contextlib import ExitStack

        with ExitStack() as ctx:
            wpool = ctx.enter_context(tc.tile_pool(name="wpool", bufs=1))
            apool = ctx.enter_context(tc.tile_pool(name="apool", bufs=8))
            qkpool = ctx.enter_context(tc.tile_pool(name="qkpool", bufs=1))
            atpool = ctx.enter_context(tc.tile_pool(name="atpool", bufs=3))
            attpool = ctx.enter_context(tc.tile_pool(name="attpool", bufs=3))
            xhpool = ctx.enter_context(tc.tile_pool(name="xhpool", bufs=2))
            xtpool = ctx.enter_context(tc.tile_pool(name="xtpool", bufs=1))
            stpool = ctx.enter_context(tc.tile_pool(name="stpool", bufs=3))
            xopool = ctx.enter_context(tc.tile_pool(name="xopool", bufs=3))
            psA = ctx.enter_context(tc.tile_pool(name="psA", bufs=2, space="PSUM"))
            psB = ctx.enter_context(tc.tile_pool(name="psB", bufs=2, space="PSUM"))
            psX = ctx.enter_context(tc.tile_pool(name="psX", bufs=2, space="PSUM"))

            # ---- constants / weights -------------------------------------
            w_s = {}
            for name, dram in (("wq", wqT), ("wk", wkT), ("wv", wvT)):
                t = wpool.tile([128, CC * 256], DT, tag=name, name=name)
                for cc in range(CC):
                    nc.sync.dma_start(t[:, cc * 256:(cc + 1) * 256],
                                      dram[cc * 128:(cc + 1) * 128, :])
                w_s[name] = t
            pj_s = []
            for g in range(2):
                t = wpool.tile([128, C], DT, tag=f"pj{g}", name=f"pj{g}")
                nc.sync.dma_start(t[:], pjT[g * 128:(g + 1) * 128, :])
                pj_s.append(t)
            if use_mask:
                mb_s = wpool.tile([128, 128], F32, tag="mb", name="mb")
                nc.sync.dma_start(mb_s[:], mb_d[:])
                mbT_s = wpool.tile([128, 128], F32, tag="mbT", name="mbT")
                nc.sync.dma_start(mbT_s[:], mbT_d[:])
            ident = wpool.tile([128, 128], BF16, tag="id", name="ident")
            make_identity(nc, ident[:])

            # ---- phase 1: projections ------------------------------------
            # qh/kh: [128 = pair(2 heads) x 64, N] transposed head outputs
            qh_s = [qkpool.tile([128, N], DT, tag=f"qh{g}", name=f"qh{g}")
                    for g in range(2)]
            kh_s = [qkpool.tile([128, N], DT, tag=f"kh{g}", name=f"kh{g}")
                    for g in range(2)]
            # vh: natural layout [key-in-block, kb * 256 + head*64 + d]
            vh_s = qkpool.tile([128, NB * 256], DT, tag="vh", name="vh")

            for tname, dram, wname in (("q", qT, "wq"), ("k", kT, "wk"),
                                       ("v", vT, "wv")):
                for nh in range(2):
                    acts = []
                    for cc in range(CC):
                        a = apool.tile([128, 1024], DT, tag="acts", name="acts")
                        nc.sync.dma_start(
                            a[:], dram[cc * 128:(cc + 1) * 128,
                                       nh * 1024:(nh + 1) * 1024])
                        acts.append(a)
                    if tname in ("q", "k"):
                        dst = qh_s if tname == "q" else kh_s
                        for g in range(2):
                            for n4 in range(2):
                                ps = psA.tile([128, 512], F32, tag="mm",
                                              name="ps_mm")
                                for cc in range(CC):
                                    nc.tensor.matmul(
                                        ps[:],
                                        w_s[wname][:, cc * 256 + g * 128:
                                                   cc * 256 + (g + 1) * 128],
                                        acts[cc][:, n4 * 512:(n4 + 1) * 512],
                                        start=(cc == 0), stop=(cc == CC - 1))
                                nc.scalar.copy(
                                    dst[g][:, nh * 1024 + n4 * 512:
                                           nh * 1024 + (n4 + 1) * 512], ps[:])
                    else:
                        for kb8 in range(8):
                            kb = nh * 8 + kb8
                            ps = psA.tile([128, 256], F32, tag="mm",
                                          name="ps_mm")
                            for cc in range(CC):
                                nc.tensor.matmul(
                                    ps[:],
                                    acts[cc][:, kb8 * 128:(kb8 + 1) * 128],
                                    w_s[wname][:, cc * 256:(cc + 1) * 256],
                                    start=(cc == 0), stop=(cc == CC - 1))
                            nc.scalar.copy(
                                vh_s[:, kb * 256:(kb + 1) * 256], ps[:])

            # ---- phase 2: attention per head -----------------------------
            xhT = [xtpool.tile([128, N], DT, tag=f"xhT{g}", name=f"xhT{g}")
                   for g in range(2)]

            for hl in range(HPC):
                g, po = hl // 2, (hl % 2) * 64
                if po == 0:
                    xh_pair = xhpool.tile([128, N], DT, tag="xhp",
                                          name="xh_pair")
                # fp32 accumulator for unnormalized attn^T @ v
                xh_acc = xhpool.tile([128, NB * 64], F32, tag="xha",
                                     name="xh_acc")
                rec_all = stpool.tile([128, NB], F32, tag="rec", name="rec_all")

                for i in range(NB):
                    nkb = (i + 1) if use_mask else NB
                    kcols = nkb * 128
                    nchunks = _ceil_div(kcols, 512)
                    qs0 = i * 128 if use_mask else 0

                    # --- S chunks -> exp -> attn row-block ---
                    sums = stpool.tile([128, 4], F32, tag="sums", name="sums")
                    attn_t = atpool.tile([128, N], ADT, tag="attn",
                                         name="attn_t")
                    for ck in range(nchunks):
                        c0 = ck * 512
                        cols = min(512, kcols - c0)
                        ps = psA.tile([128, 512], F32, tag="mm", name="ps_mm")
                        nc.tensor.matmul(
                            ps[:, :cols],
                            qh_s[g][po:po + 64, i * 128:(i + 1) * 128],
                            kh_s[g][po:po + 64, c0:c0 + cols],
                            start=True, stop=True)
                        if use_mask and ck == nchunks - 1:
                            d0 = i * 128 - c0
                            nc.vector.tensor_add(ps[:, d0:d0 + 128],
                                                 ps[:, d0:d0 + 128], mb_s[:])
                        nc.scalar.activation(
                            attn_t[:, c0:c0 + cols], ps[:, :cols], ACTF.Exp,
                            scale=SCALE, accum_out=sums[:, ck:ck + 1])
                    if nchunks > 1:
                        tot = stpool.tile([128, 1], F32, tag="tot", name="tot")
                        nc.vector.tensor_reduce(tot[:], sums[:, :nchunks],
                                                axis=AX.X, op=ALU.add)
                    else:
                        tot = sums[:, 0:1]
                    nc.vector.reciprocal(rec_all[:, i:i + 1], tot[:])
                    nc.vector.tensor_scalar_mul(attn_t[:, :kcols],
                                                attn_t[:, :kcols],
                                                rec_all[:, i:i + 1])
                    nc.sync.dma_start(attn_o[hl, i * 128:(i + 1) * 128, 0:kcols],
                                      attn_t[:, :kcols])

                    # --- S^T chunks -> exp (unnormalized attnT, bf16) ---
                    attnT_t = attpool.tile([128, N], DT, tag="attnT",
                                           name="attnT_t")
                    qcols_tot = N - qs0
                    for ck in range(_ceil_div(qcols_tot, 512)):
                        c0 = qs0 + ck * 512
                        cols = min(512, N - c0)
                        ps = psB.tile([128, 512], F32, tag="st", name="ps_st")
                        nc.tensor.matmul(
                            ps[:, :cols],
                            kh_s[g][po:po + 64, i * 128:(i + 1) * 128],
                            qh_s[g][po:po + 64, c0:c0 + cols],
                            start=True, stop=True)
                        if use_mask and ck == 0:
                            nc.vector.tensor_add(ps[:, 0:128], ps[:, 0:128],
                                                 mbT_s[:])
                        nc.scalar.activation(attnT_t[:, c0:c0 + cols],
                                             ps[:, :cols], ACTF.Exp,
                                             scale=SCALE)

                    # --- attn^T @ v contributions for q-blocks >= i ---
                    i2lo = i if use_mask else 0
                    nreg = NB - i2lo
                    avp = psX.tile([128, NB * 64], F32, tag="av", name="av_ps")
                    for i2 in range(i2lo, NB):
                        nc.tensor.matmul(
                            avp[:, i2 * 64:(i2 + 1) * 64],
                            attnT_t[:, i2 * 128:(i2 + 1) * 128],
                            vh_s[:, i * 256 + hl * 64:i * 256 + hl * 64 + 64],
                            start=True, stop=True)
                    if i == 0:
                        nc.vector.tensor_copy(xh_acc[:, i2lo * 64:],
                                              avp[:, i2lo * 64:])
                    else:
                        nc.vector.tensor_add(xh_acc[:, i2lo * 64:],
                                             xh_acc[:, i2lo * 64:],
                                             avp[:, i2lo * 64:])
                    if use_mask:
                        nc.vector.tensor_scalar_mul(
                            xh_pair[:, i * 128 + po:i * 128 + po + 64],
                            xh_acc[:, i * 64:(i + 1) * 64], rec_all[:, i:i + 1])
                if not use_mask:
                    for i2 in range(NB):
                        nc.vector.tensor_scalar_mul(
                            xh_pair[:, i2 * 128 + po:i2 * 128 + po + 64],
                            xh_acc[:, i2 * 64:(i2 + 1) * 64],
                            rec_all[:, i2:i2 + 1])

                if po == 64:  # pair complete -> transpose into xhT[g]
                    for ib in range(NB):
                        tp = psB.tile([128, 128], BF16, tag="st", name="ps_tp")
                        nc.tensor.transpose(tp[:],
                                            xh_pair[:, ib * 128:(ib + 1) * 128],
                                            ident[:])
                        nc.vector.tensor_copy(xhT[g][:, ib * 128:(ib + 1) * 128],
                                              tp[:])

            # ---- phase 3: output projection (partial over local heads) ---
            for nb in range(NB):
                for oc in range(2):
                    ps = psA.tile([128, 512], F32, tag="mm", name="ps_mm")
                    for g in range(2):
                        nc.tensor.matmul(ps[:],
                                         xhT[g][:, nb * 128:(nb + 1) * 128],
                                         pj_s[g][:, oc * 512:(oc + 1) * 512],
                                         start=(g == 0), stop=(g == 1))
                    xo = xopool.tile([128, 512], F32, tag="xout", name="xo")
                    nc.vector.tensor_copy(xo[:], ps[:])
                    nc.sync.dma_start(x_o[nb * 128:(nb + 1) * 128,
                                          oc * 512:(oc + 1) * 512], xo[:])

    print(f"[kernel] trace+schedule: {_time.time() - _t0:.1f}s", flush=True)
    _t1 = _time.time()
    nc.compile()
    print(f"[kernel] bacc compile: {_time.time() - _t1:.1f}s", flush=True)
    return nc


def _get_nc(use_mask: bool):
    key = bool(use_mask)
    if key not in _cache:
        _cache[key] = build_kernel(key)
    return _cache[key]


def kernel(q, k, v, qkv_w, proj_w, proj_b, use_mask):
    global last_exec_time_ns, last_results
    q = np.asarray(q, dtype=np.float32)
    k = np.asarray(k, dtype=np.float32)
    v = np.asarray(v, dtype=np.float32)
    qkv_w = np.asarray(qkv_w, dtype=np.float32)
    proj_w = np.asarray(proj_w, dtype=np.float32)
    proj_b = np.asarray(proj_b, dtype=np.float32)
    um = bool(int(np.asarray(use_mask)))

    nc = _get_nc(um)

    wq, wk, wv = qkv_w[:C], qkv_w[C:2 * C], qkv_w[2 * C:]
    qTs = [np.ascontiguousarray(q[b].T).astype(NPBF) for b in range(B)]
    kTs = [np.ascontiguousarray(k[b].T).astype(NPBF) for b in range(B)]
    vTs = [np.ascontiguousarray(v[b].T).astype(NPBF) for b in range(B)]

    r = np.arange(128)
    maskb = np.where(r[None, :] <= r[:, None], 0.0, MASK_VAL).astype(np.float32)
    maskbT = np.ascontiguousarray(maskb.T)

    in_maps = []
    for c in range(N_CORES):
        b, h0 = c // HPC, (c % HPC) * HPC
        sl = slice(h0 * HD, (h0 + HPC) * HD)
        in_maps.append({
            "qT": qTs[b], "kT": kTs[b], "vT": vTs[b],
            "wqT": np.ascontiguousarray(wq[sl].T).astype(NPBF),
            "wkT": np.ascontiguousarray(wk[sl].T).astype(NPBF),
            "wvT": np.ascontiguousarray(wv[sl].T).astype(NPBF),
            "pjT": np.ascontiguousarray(proj_w[:, sl].T).astype(NPBF),
            "maskb": maskb, "maskbT": maskbT,
        })

    trace = os.environ.get("BASS_ATTN_TRACE", "0") == "1"
    import time as _time
    _t0 = _time.time()
    res = run_bass_kernel_spmd(nc, in_maps, core_ids=list(range(N_CORES)),
                               trace=trace)
    print(f"[kernel] spmd run (compile+exec): {_time.time() - _t0:.1f}s",
          flush=True)
    last_exec_time_ns = res.exec_time_ns
    last_results = res

    attn_full = np.zeros((B, H, N, N), dtype=np.float32)
    x_full = np.zeros((B, N, C), dtype=np.float32)
    for c in range(N_CORES):
        b, h0 = c // HPC, (c % HPC) * HPC
        a = res.results[c]["attn"]
        if um:
            for i in range(NB):
                r1 = (i + 1) * 128
                attn_full[b, h0:h0 + HPC, i * 128:r1, :r1] = \
                    a[:, i * 128:r1, :r1].astype(np.float32)
        else:
            attn_full[b, h0:h0 + HPC] = a.astype(np.float32)
        x_full[b] += res.results[c]["x"]
    x_full += proj_b[None, None, :]
    return x_full, attn_full


# revision 6
# speedup vs baseline: 1.1706x; 1.1706x over previous
"""Trainium2 8-core Bass kernel for fused-QKV causal attention.

Problem: q,k,v [2,2048,1024] f32; fused qkv_w [3072,1024]; proj_w [1024,1024];
returns (x [2,2048,1024], attn [2,16,2048,2048]) matching the jax reference.

Sharding: 8 cores = 2 batches x 4 head-groups (4 heads each). Host transposes
activations/weights so device DMAs are contiguous; host sums the 4 per-core
output-projection partials per batch (the only cross-core reduction).

Matmul inputs are bf16 (PE runs 1 cyc/row vs fp32's 2x2 half-speed passes);
all accumulation/softmax statistics stay fp32.
"""
import os
import sys

sys.path.insert(0, "/opt/trn_rl_repo")

# The image's antenv lacks axon_hooks; fabricate it so trace=True works.
import types as _types

if "antenv.axon_hooks" not in sys.modules:
    _hooks = _types.ModuleType("antenv.axon_hooks")
    _h = [None]
    _hooks.set_axon_ntff_profile_hook = lambda h: _h.__setitem__(0, h)
    _hooks.get_axon_ntff_profile_hook = lambda: _h[0]
    sys.modules["antenv.axon_hooks"] = _hooks
    try:
        from trn_agent_boot.trn_boot import _ntff_profile_via_ctypes

        _hooks.set_axon_ntff_profile_hook(
            _ntff_profile_via_ctypes("/opt/axon/libaxon_pjrt.so")
        )
    except Exception:
        pass

import ml_dtypes
import numpy as np

import concourse.bass as bass
import concourse.mybir as mybir
import concourse.tile as tile
from concourse import bacc
from concourse.bass_utils import run_bass_kernel_spmd
from concourse.masks import make_identity

F32 = mybir.dt.float32
BF16 = mybir.dt.bfloat16
AX = mybir.AxisListType
ALU = mybir.AluOpType
ACTF = mybir.ActivationFunctionType
NPBF = ml_dtypes.bfloat16

B, N, C = 2, 2048, 1024
H, HD = 16, 64
SCALE = HD ** -0.5
HPC = 4            # heads per core
N_CORES = 8
NB = N // 128      # 16 query/key blocks
CC = C // 128      # 8 contraction chunks
MASK_VAL = -30000.0
ATTN_OUT_BF16 = True   # device writes attn probs as bf16; host casts to f32

_cache = {}
last_exec_time_ns = None
last_results = None


def _ceil_div(a, b):
    return (a + b - 1) // b


def build_kernel(use_mask: bool):
    import time as _time
    _t0 = _time.time()
    nc = bacc.Bacc("TRN2", target_bir_lowering=False, debug=False,
                   num_devices=N_CORES)

    DT = BF16
    qT = nc.dram_tensor("qT", [C, N], DT, kind="ExternalInput").ap()
    kT = nc.dram_tensor("kT", [C, N], DT, kind="ExternalInput").ap()
    vT = nc.dram_tensor("vT", [C, N], DT, kind="ExternalInput").ap()
    wqT = nc.dram_tensor("wqT", [C, HPC * HD], DT, kind="ExternalInput").ap()
    wkT = nc.dram_tensor("wkT", [C, HPC * HD], DT, kind="ExternalInput").ap()
    wvT = nc.dram_tensor("wvT", [C, HPC * HD], DT, kind="ExternalInput").ap()
    pjT = nc.dram_tensor("pjT", [HPC * HD, C], DT, kind="ExternalInput").ap()
    mb_d = nc.dram_tensor("maskb", [128, 128], BF16, kind="ExternalInput").ap()
    mbT_d = nc.dram_tensor("maskbT", [128, 128], BF16, kind="ExternalInput").ap()

    ADT = BF16 if ATTN_OUT_BF16 else F32
    attn_o = nc.dram_tensor("attn", [HPC, N, N], ADT, kind="ExternalOutput").ap()
    x_o = nc.dram_tensor("x", [N, C], F32, kind="ExternalOutput").ap()

    with tile.TileContext(nc) as tc:
        from contextlib import ExitStack

        with ExitStack() as ctx:
            wpool = ctx.enter_context(tc.tile_pool(name="wpool", bufs=1))
            apool = ctx.enter_context(tc.tile_pool(name="apool", bufs=8))
            qkpool = ctx.enter_context(tc.tile_pool(name="qkpool", bufs=1))
            atpool = ctx.enter_context(tc.tile_pool(name="atpool", bufs=3))
            attpool = ctx.enter_context(tc.tile_pool(name="attpool", bufs=3))
            xhpool = ctx.enter_context(tc.tile_pool(name="xhpool", bufs=2))
            xtpool = ctx.enter_context(tc.tile_pool(name="xtpool", bufs=1))
            stpool = ctx.enter_context(tc.tile_pool(name="stpool", bufs=3))
            xopool = ctx.enter_context(tc.tile_pool(name="xopool", bufs=3))
            psA = ctx.enter_context(tc.tile_pool(name="psA", bufs=2, space="PSUM"))
            psB = ctx.enter_context(tc.tile_pool(name="psB", bufs=2, space="PSUM"))
            psX = ctx.enter_context(tc.tile_pool(name="psX", bufs=2, space="PSUM"))

            # ---- constants / weights -------------------------------------
            w_s = {}
            for name, dram in (("wq", wqT), ("wk", wkT), ("wv", wvT)):
                t = wpool.tile([128, CC * 256], DT, tag=name, name=name)
                for cc in range(CC):
                    nc.sync.dma_start(t[:, cc * 256:(cc + 1) * 256],
                                      dram[cc * 128:(cc + 1) * 128, :])
                w_s[name] = t
            pj_s = []
            for g in range(2):
                t = wpool.tile([128, C], DT, tag=f"pj{g}", name=f"pj{g}")
                nc.sync.dma_start(t[:], pjT[g * 128:(g + 1) * 128, :])
                pj_s.append(t)
            if use_mask:
                mb_s = wpool.tile([128, 128], BF16, tag="mb", name="mb")
                nc.sync.dma_start(mb_s[:], mb_d[:])
                mbT_s = wpool.tile([128, 128], BF16, tag="mbT", name="mbT")
                nc.sync.dma_start(mbT_s[:], mbT_d[:])
            ident = wpool.tile([128, 128], BF16, tag="id", name="ident")
            make_identity(nc, ident[:])

            # ---- phase 1: projections ------------------------------------
            # qh/kh: [128 = pair(2 heads) x 64, N] transposed head outputs
            qh_s = [qkpool.tile([128, N], DT, tag=f"qh{g}", name=f"qh{g}")
                    for g in range(2)]
            kh_s = [qkpool.tile([128, N], DT, tag=f"kh{g}", name=f"kh{g}")
                    for g in range(2)]
            # vh: natural layout [key-in-block, kb * 256 + head*64 + d]
            vh_s = qkpool.tile([128, NB * 256], DT, tag="vh", name="vh")

            for tname, dram, wname in (("q", qT, "wq"), ("k", kT, "wk"),
                                       ("v", vT, "wv")):
                for nh in range(2):
                    acts = []
                    for cc in range(CC):
                        a = apool.tile([128, 1024], DT, tag="acts", name="acts")
                        nc.sync.dma_start(
                            a[:], dram[cc * 128:(cc + 1) * 128,
                                       nh * 1024:(nh + 1) * 1024])
                        acts.append(a)
                    if tname in ("q", "k"):
                        dst = qh_s if tname == "q" else kh_s
                        for g in range(2):
                            for n4 in range(2):
                                ps = psA.tile([128, 512], F32, tag="mm",
                                              name="ps_mm")
                                for cc in range(CC):
                                    nc.tensor.matmul(
                                        ps[:],
                                        w_s[wname][:, cc * 256 + g * 128:
                                                   cc * 256 + (g + 1) * 128],
                                        acts[cc][:, n4 * 512:(n4 + 1) * 512],
                                        start=(cc == 0), stop=(cc == CC - 1))
                                nc.vector.tensor_copy(
                                    dst[g][:, nh * 1024 + n4 * 512:
                                           nh * 1024 + (n4 + 1) * 512], ps[:])
                    else:
                        for kb8 in range(8):
                            kb = nh * 8 + kb8
                            ps = psA.tile([128, 256], F32, tag="mm",
                                          name="ps_mm")
                            for cc in range(CC):
                                nc.tensor.matmul(
                                    ps[:],
                                    acts[cc][:, kb8 * 128:(kb8 + 1) * 128],
                                    w_s[wname][:, cc * 256:(cc + 1) * 256],
                                    start=(cc == 0), stop=(cc == CC - 1))
                            nc.vector.tensor_copy(
                                vh_s[:, kb * 256:(kb + 1) * 256], ps[:])

            # ---- phase 2: attention per head -----------------------------
            xhT = [xtpool.tile([128, N], DT, tag=f"xhT{g}", name=f"xhT{g}")
                   for g in range(2)]

            for hl in range(HPC):
                g, po = hl // 2, (hl % 2) * 64
                if po == 0:
                    xh_pair = xhpool.tile([128, N], DT, tag="xhp",
                                          name="xh_pair")
                # fp32 accumulator for unnormalized attn^T @ v
                xh_acc = xhpool.tile([128, NB * 64], F32, tag="xha",
                                     name="xh_acc")
                rec_all = stpool.tile([128, NB], F32, tag="rec", name="rec_all")

                for i in range(NB):
                    nkb = (i + 1) if use_mask else NB
                    kcols = nkb * 128
                    nchunks = _ceil_div(kcols, 512)
                    qs0 = i * 128 if use_mask else 0

                    # --- S chunks -> exp -> attn row-block ---
                    attn_t = atpool.tile([128, N], ADT, tag="attn",
                                         name="attn_t")
                    for ck in range(nchunks):
                        c0 = ck * 512
                        cols = min(512, kcols - c0)
                        ps = psA.tile([128, 512], F32, tag="mm", name="ps_mm")
                        nc.tensor.matmul(
                            ps[:, :cols],
                            qh_s[g][po:po + 64, i * 128:(i + 1) * 128],
                            kh_s[g][po:po + 64, c0:c0 + cols],
                            start=True, stop=True)
                        nc.scalar.activation(
                            attn_t[:, c0:c0 + cols], ps[:, :cols], ACTF.Exp,
                            scale=SCALE)
                    if use_mask:
                        d0 = i * 128
                        nc.vector.tensor_mult(attn_t[:, d0:d0 + 128],
                                              attn_t[:, d0:d0 + 128], mb_s[:])
                    tot = stpool.tile([128, 1], F32, tag="tot", name="tot")
                    nc.vector.tensor_reduce(tot[:], attn_t[:, :kcols],
                                            axis=AX.X, op=ALU.add)
                    nc.vector.reciprocal(rec_all[:, i:i + 1], tot[:])
                    nc.vector.tensor_scalar_mul(attn_t[:, :kcols],
                                                attn_t[:, :kcols],
                                                rec_all[:, i:i + 1])
                    nc.sync.dma_start(attn_o[hl, i * 128:(i + 1) * 128, 0:kcols],
                                      attn_t[:, :kcols])

                    # --- S^T chunks -> exp (unnormalized attnT, bf16) ---
                    attnT_t = attpool.tile([128, N], DT, tag="attnT",
                                           name="attnT_t")
                    qcols_tot = N - qs0
                    for ck in range(_ceil_div(qcols_tot, 512)):
                        c0 = qs0 + ck * 512
                        cols = min(512, N - c0)
                        ps = psB.tile([128, 512], F32, tag="st", name="ps_st")
                        nc.tensor.matmul(
                            ps[:, :cols],
                            kh_s[g][po:po + 64, i * 128:(i + 1) * 128],
                            qh_s[g][po:po + 64, c0:c0 + cols],
                            start=True, stop=True)
                        nc.scalar.activation(attnT_t[:, c0:c0 + cols],
                                             ps[:, :cols], ACTF.Exp,
                                             scale=SCALE)
                    if use_mask:
                        nc.vector.tensor_mult(attnT_t[:, qs0:qs0 + 128],
                                              attnT_t[:, qs0:qs0 + 128],
                                              mbT_s[:])

                    # --- attn^T @ v contributions for q-blocks >= i ---
                    i2lo = i if use_mask else 0
                    nreg = NB - i2lo
                    avp = psX.tile([128, NB * 64], F32, tag="av", name="av_ps")
                    for i2 in range(i2lo, NB):
                        nc.tensor.matmul(
                            avp[:, i2 * 64:(i2 + 1) * 64],
                            attnT_t[:, i2 * 128:(i2 + 1) * 128],
                            vh_s[:, i * 256 + hl * 64:i * 256 + hl * 64 + 64],
                            start=True, stop=True)
                    if i == 0:
                        nc.vector.tensor_copy(xh_acc[:, i2lo * 64:],
                                              avp[:, i2lo * 64:])
                    else:
                        nc.vector.tensor_add(xh_acc[:, i2lo * 64:],
                                             xh_acc[:, i2lo * 64:],
                                             avp[:, i2lo * 64:])
                    if use_mask:
                        nc.vector.tensor_scalar_mul(
                            xh_pair[:, i * 128 + po:i * 128 + po + 64],
                            xh_acc[:, i * 64:(i + 1) * 64], rec_all[:, i:i + 1])
                if not use_mask:
                    for i2 in range(NB):
                        nc.vector.tensor_scalar_mul(
                            xh_pair[:, i2 * 128 + po:i2 * 128 + po + 64],
                            xh_acc[:, i2 * 64:(i2 + 1) * 64],
                            rec_all[:, i2:i2 + 1])

                if po == 64:  # pair complete -> transpose into xhT[g]
                    for ib in range(NB):
                        tp = psB.tile([128, 128], BF16, tag="st", name="ps_tp")
                        nc.tensor.transpose(tp[:],
                                            xh_pair[:, ib * 128:(ib + 1) * 128],
                                            ident[:])
                        nc.vector.tensor_copy(xhT[g][:, ib * 128:(ib + 1) * 128],
                                              tp[:])

            # ---- phase 3: output projection (partial over local heads) ---
            for nb in range(NB):
                for oc in range(2):
                    ps = psA.tile([128, 512], F32, tag="mm", name="ps_mm")
                    for g in range(2):
                        nc.tensor.matmul(ps[:],
                                         xhT[g][:, nb * 128:(nb + 1) * 128],
                                         pj_s[g][:, oc * 512:(oc + 1) * 512],
                                         start=(g == 0), stop=(g == 1))
                    xo = xopool.tile([128, 512], F32, tag="xout", name="xo")
                    nc.vector.tensor_copy(xo[:], ps[:])
                    nc.sync.dma_start(x_o[nb * 128:(nb + 1) * 128,
                                          oc * 512:(oc + 1) * 512], xo[:])

    print(f"[kernel] trace+schedule: {_time.time() - _t0:.1f}s", flush=True)
    _t1 = _time.time()
    nc.compile()
    print(f"[kernel] bacc compile: {_time.time() - _t1:.1f}s", flush=True)
    return nc


def _get_nc(use_mask: bool):
    key = bool(use_mask)
    if key not in _cache:
        _cache[key] = build_kernel(key)
    return _cache[key]


def kernel(q, k, v, qkv_w, proj_w, proj_b, use_mask):
    global last_exec_time_ns, last_results
    q = np.asarray(q, dtype=np.float32)
    k = np.asarray(k, dtype=np.float32)
    v = np.asarray(v, dtype=np.float32)
    qkv_w = np.asarray(qkv_w, dtype=np.float32)
    proj_w = np.asarray(proj_w, dtype=np.float32)
    proj_b = np.asarray(proj_b, dtype=np.float32)
    um = bool(int(np.asarray(use_mask)))

    nc = _get_nc(um)

    wq, wk, wv = qkv_w[:C], qkv_w[C:2 * C], qkv_w[2 * C:]
    qTs = [np.ascontiguousarray(q[b].T).astype(NPBF) for b in range(B)]
    kTs = [np.ascontiguousarray(k[b].T).astype(NPBF) for b in range(B)]
    vTs = [np.ascontiguousarray(v[b].T).astype(NPBF) for b in range(B)]

    r = np.arange(128)
    maskb = np.where(r[None, :] <= r[:, None], 1.0, 0.0).astype(NPBF)
    maskbT = np.ascontiguousarray(maskb.T)

    in_maps = []
    for c in range(N_CORES):
        b, h0 = c // HPC, (c % HPC) * HPC
        sl = slice(h0 * HD, (h0 + HPC) * HD)
        in_maps.append({
            "qT": qTs[b], "kT": kTs[b], "vT": vTs[b],
            "wqT": np.ascontiguousarray(wq[sl].T).astype(NPBF),
            "wkT": np.ascontiguousarray(wk[sl].T).astype(NPBF),
            "wvT": np.ascontiguousarray(wv[sl].T).astype(NPBF),
            "pjT": np.ascontiguousarray(proj_w[:, sl].T).astype(NPBF),
            "maskb": maskb, "maskbT": maskbT,
        })

    trace = os.environ.get("BASS_ATTN_TRACE", "0") == "1"
    import time as _time
    _t0 = _time.time()
    res = run_bass_kernel_spmd(nc, in_maps, core_ids=list(range(N_CORES)),
                               trace=trace)
    print(f"[kernel] spmd run (compile+exec): {_time.time() - _t0:.1f}s",
          flush=True)
    last_exec_time_ns = res.exec_time_ns
    last_results = res

    attn_full = np.zeros((B, H, N, N), dtype=np.float32)
    x_full = np.zeros((B, N, C), dtype=np.float32)
    for c in range(N_CORES):
        b, h0 = c // HPC, (c % HPC) * HPC
        a = res.results[c]["attn"]
        if um:
            for i in range(NB):
                r1 = (i + 1) * 128
                attn_full[b, h0:h0 + HPC, i * 128:r1, :r1] = \
                    a[:, i * 128:r1, :r1].astype(np.float32)
        else:
            attn_full[b, h0:h0 + HPC] = a.astype(np.float32)
        x_full[b] += res.results[c]["x"]
    x_full += proj_b[None, None, :]
    return x_full, attn_full
